# revision 11
# baseline (speedup 1.0000x reference)
"""Trainium2 Bass kernel for DCANet head (nn_DCANet_1795296329879).

Data-parallel over num_qry: 75 queries -> 8 cores x 10 (core 7 padded with
duplicates). Support features + SE weights replicated. No collectives.

Math per core (NQL=10 local queries, WAY=5, C=640, HW=100):
  x = x - mean_c(x)                      (channel mean per spatial pos)
  x = x * sigmoid(W2 relu(W1 mean_s(x) + b1) + b2)   (squeeze-excitation)
  n = x / ||x||_c                        (L2 over channels)
  corr[q,w,s,l] = <n_spt[w,:,s], n_qry[q,:,l]>
  cs = softmax_s(gauss_norm_s(corr)/5);  attn_s = sum_l cs
  cq = softmax_l(gauss_norm_l(corr)/5);  attn_q = sum_s cq
  sp[q,w,:] = spt[w] @ attn_s[q,w];  qp[q,w,:] = qry[q] @ attn_q[q,w]
  out[q,w] = cos(sp,qp) / 0.2
"""

import numpy as np

NQL = 10          # queries per core
WAY = 5
C = 640
KC = 5            # C = KC * 128 chunks
HW = 100
NQ = 75
N_CORES = 8

K1 = 2500.0 / 99.0    # var scale:  25 * (100/99)  (unbiased var + /TEMP_ATTN^2)
K2 = 2.5e-4           # 25 * eps(1e-5)


def _build():
    import concourse.bass as bass
    import concourse.tile as tile
    from concourse import bacc, mybir
    from concourse.masks import make_identity

    f32 = mybir.dt.float32
    f32r = mybir.dt.float32r
    bf16 = mybir.dt.bfloat16
    AF = mybir.ActivationFunctionType
    ALU = mybir.AluOpType
    AX = mybir.AxisListType

    nc = bacc.Bacc("TRN2", target_bir_lowering=False, debug=False,
                   num_devices=N_CORES)

    qry_d = nc.dram_tensor("qry", [NQL, C, HW], f32, kind="ExternalInput").ap()
    spt_d = nc.dram_tensor("spt", [WAY, C, HW], f32, kind="ExternalInput").ap()
    w1t_d = nc.dram_tensor("w1t", [C, 40], f32, kind="ExternalInput").ap()
    w2t_d = nc.dram_tensor("w2t", [40, C], f32, kind="ExternalInput").ap()
    b1_d = nc.dram_tensor("b1", [40, 1], f32, kind="ExternalInput").ap()
    nb2_d = nc.dram_tensor("nb2", [C], f32, kind="ExternalInput").ap()
    out_d = nc.dram_tensor("out", [WAY * NQL, 1], f32, kind="ExternalOutput").ap()

    with tile.TileContext(nc) as tc:
        _body(nc, tc, mybir, bass, make_identity, f32, f32r, bf16, AF, ALU, AX,
              qry_d, spt_d, w1t_d, w2t_d, b1_d, nb2_d, out_d)
    nc.compile()
    return nc


def _body(nc, tc, mybir, bass, make_identity, f32, f32r, bf16, AF, ALU, AX,
          qry_d, spt_d, w1t_d, w2t_d, b1_d, nb2_d, out_d):
    from contextlib import ExitStack

    ctx = ExitStack()
    with ctx:
        P = ctx.enter_context(tc.tile_pool(name="persist", bufs=1))
        S = ctx.enter_context(tc.tile_pool(name="scratch", bufs=2))

        # ---- persistent SBUF tiles ----
        qry_sb = P.tile([128, KC, NQL, HW], f32)      # becomes x_se in place
        spt_sb = P.tile([128, KC, WAY, HW], f32)
        snq_sb = P.tile([128, KC, NQL, HW], f32)      # L2-normalized qry
        sns_sb = P.tile([128, KC, WAY, HW], f32)      # L2-normalized spt
        w1t_sb = P.tile([128, KC, 40], f32)
        w2t_sb = P.tile([40, C], f32)
        b1_sb = P.tile([40, 1], f32)
        nb2_sb = P.tile([128, KC], f32)
        mbq_sb = P.tile([128, NQL, HW], f32)          # qry chan-mean bcast
        mbs_sb = P.tile([128, WAY, HW], f32)
        rsq_sb = P.tile([128, NQL, HW], f32)          # 1/||qry|| bcast
        rss_sb = P.tile([128, WAY, HW], f32)
        sall_sb = P.tile([128, KC, NQL + WAY], f32)   # SE pooled feats
        g_sb = P.tile([128, KC, NQL + WAY], f32)      # SE gates
        h_sb = P.tile([40, NQL + WAY], f32)
        xTq_sb = P.tile([HW, NQL, C], f32)            # qry transposed [l, c]
        xTs_sb = P.tile([HW, WAY, C], f32)            # spt transposed [s, c]
        expq_sb = P.tile([HW, NQL, WAY, HW], f32)     # exp tiles, cs path
        exps_sb = P.tile([HW, WAY, NQL, HW], f32)     # exp tiles, cq path
        alphaq_sb = P.tile([HW, NQL, WAY], f32)       # cs inv-std/5
        alphas_sb = P.tile([HW, WAY, NQL], f32)       # cq inv-std/5
        rq_sb = P.tile([HW, NQL, WAY], f32)           # cs softmax 1/denom
        rs_sb = P.tile([HW, WAY, NQL], f32)           # cq softmax 1/denom
        attn_sT_sb = P.tile([HW, WAY, NQL], f32)
        attn_qT_sb = P.tile([HW, NQL, WAY], f32)
        sp_all = P.tile([128, KC, NQL, WAY], f32)     # [c, k, q, w]
        qp_all = P.tile([128, KC, NQL, WAY], f32)
        prod_sb = P.tile([128, KC * NQL * WAY], f32)
        fin_sb = P.tile([1, 8, NQL * WAY], f32)       # num/ss/qq/den/../sim
        qbf_sb = P.tile([128, KC, NQL, HW], bf16)
        sbf_sb = P.tile([128, KC, WAY, HW], bf16)
        onesbf = P.tile([128, 128], bf16)
        ones1 = P.tile([128, 128], f32)
        ident = P.tile([128, 128], f32)
        epsc = P.tile([128, 1], f32)
        k2c = P.tile([128, 1], f32)

        nc.vector.memset(onesbf, 1.0)
        nc.vector.memset(ones1, 1.0)
        nc.vector.memset(epsc, 1e-30)
        nc.vector.memset(k2c, K2)
        make_identity(nc, ident)

        # ---- input DMAs ----
        qry_dv = qry_d.rearrange("n (k p) s -> p k n s", p=128)
        spt_dv = spt_d.rearrange("n (k p) s -> p k n s", p=128)
        for k in range(KC):
            nc.sync.dma_start(qry_sb[:, k], qry_dv[:, k])
            nc.sync.dma_start(spt_sb[:, k], spt_dv[:, k])
        nc.sync.dma_start(w1t_sb,
                          w1t_d.rearrange("(k p) j -> p k j", p=128))
        nc.sync.dma_start(w2t_sb, w2t_d)
        nc.sync.dma_start(b1_sb, b1_d)
        nc.sync.dma_start(nb2_sb, nb2_d.rearrange("(k p) -> p k", p=128))

        qv = qry_sb
        sv = spt_sb

        # ================= preprocessing =================
        # bf16 shadows (for sum-matmuls; stats don't need full precision)
        for k in range(KC):
            nc.gpsimd.tensor_copy(qbf_sb[:, k], qv[:, k])
            nc.vector.tensor_copy(sbf_sb[:, k], sv[:, k])

        # channel-mean (replicated over partitions via ones matmul)
        with tc.tile_pool(name="ps_pre", bufs=1, space="PSUM") as PP:
            mbq_ps = [PP.tile([128, 500], f32, tag=f"mbq{h}", name=f"mbq_ps{h}") for h in range(2)]
            mbs_ps = PP.tile([128, WAY * HW], f32, tag="mbs")   # 1 bank
            for h in range(2):
                for k in range(KC):
                    nc.tensor.matmul(
                        mbq_ps[h],
                        onesbf,
                        qbf_sb[:, k, 5 * h:5 * h + 5, :],
                        start=(k == 0), stop=(k == KC - 1))
            for k in range(KC):
                nc.tensor.matmul(mbs_ps, onesbf,
                                 sbf_sb[:, k, :, :],
                                 start=(k == 0), stop=(k == KC - 1))
            mbq_f = mbq_sb.rearrange("p n s -> p (n s)")
            nc.scalar.mul(mbq_f[:, 0:500], mbq_ps[0], 1.0 / C)
            nc.scalar.mul(mbq_f[:, 500:1000], mbq_ps[1], 1.0 / C)
            nc.scalar.mul(mbs_sb.rearrange("p n s -> p (n s)"), mbs_ps, 1.0 / C)

        # subtract channel mean (qry on gpsimd, spt on vector)
        for k in range(KC):
            nc.gpsimd.tensor_tensor(qv[:, k], qv[:, k], mbq_sb,
                                    ALU.subtract)
            nc.vector.tensor_tensor(sv[:, k], sv[:, k], mbs_sb,
                                    ALU.subtract)

        # SE: spatial sum -> fc1 -> relu -> fc2 -> sigmoid
        nc.vector.tensor_reduce(sall_sb[:, :, :NQL], qv, AX.X, ALU.add)
        nc.vector.tensor_reduce(sall_sb[:, :, NQL:], sv, AX.X, ALU.add)

        with tc.tile_pool(name="ps_se", bufs=1, space="PSUM") as PS:
            h_ps = PS.tile([40, NQL + WAY], f32, tag="h")
            g_ps = PS.tile([128, KC, NQL + WAY], f32, tag="g")
            for k in range(KC):
                nc.tensor.matmul(h_ps, w1t_sb[:, k, :],
                                 sall_sb[:, k, :],
                                 start=(k == 0), stop=(k == KC - 1))
            nc.scalar.activation(h_sb, h_ps, AF.Relu,
                                 bias=b1_sb[:, 0:1], scale=1.0 / HW)
            for k in range(KC):
                nc.tensor.matmul(g_ps[:, k, :],
                                 w2t_sb[:, 128 * k:128 * (k + 1)],
                                 h_sb, start=True, stop=True)
                # sigmoid(x) = 1/(1+exp(-x)); bias holds -b2
                nc.scalar.activation(g_sb[:, k, :], g_ps[:, k, :],
                                     AF.Exp, bias=nb2_sb[:, k:k + 1],
                                     scale=-1.0)
        nc.vector.tensor_scalar_add(g_sb, g_sb, 1.0)
        nc.vector.reciprocal(g_sb, g_sb)

        # apply SE gates
        for k in range(KC):
            nc.gpsimd.tensor_tensor(
                qv[:, k], qv[:, k],
                g_sb[:, k, :NQL].to_broadcast((128, NQL, HW)), ALU.mult)
            nc.vector.tensor_tensor(
                sv[:, k], sv[:, k],
                g_sb[:, k, NQL:].to_broadcast((128, WAY, HW)), ALU.mult)

        # L2 norms over channels: rsq = (sum_c x^2)^-0.5 (replicated bcast)
        with tc.tile_pool(name="ps_ss", bufs=1, space="PSUM") as PN:
            ssq_ps = [PN.tile([128, 500], f32, tag=f"ssq{h}", name=f"ssq_ps{h}") for h in range(2)]
            sss_ps = PN.tile([128, WAY * HW], f32, tag="sss")
            for k in range(KC):
                x2q = S.tile([128, NQL, HW], bf16, tag="x2q")
                nc.vector.tensor_tensor(x2q, qv[:, k], qv[:, k], ALU.mult)
                for h in range(2):
                    nc.tensor.matmul(
                        ssq_ps[h],
                        onesbf,
                        x2q.rearrange("p n s -> p (n s)")[:, h * 500:(h + 1) * 500],
                        start=(k == 0), stop=(k == KC - 1))
                x2s = S.tile([128, WAY, HW], bf16, tag="x2s")
                nc.gpsimd.tensor_tensor(x2s, sv[:, k], sv[:, k], ALU.mult)
                nc.tensor.matmul(sss_ps, onesbf,
                                 x2s.rearrange("p n s -> p (n s)"),
                                 start=(k == 0), stop=(k == KC - 1))
            # rs = exp(-0.5 * ln(ss)): avoids sqrt's table set
            lnq = S.tile([128, NQL * HW], f32, tag="lnq")
            lns = S.tile([128, WAY * HW], f32, tag="lns")
            for h in range(2):
                nc.scalar.activation(lnq[:, 500 * h:500 * (h + 1)], ssq_ps[h],
                                     AF.Ln, bias=epsc, scale=1.0)
            nc.scalar.activation(lns, sss_ps, AF.Ln,
                                 bias=epsc, scale=1.0)
            nc.scalar.activation(rsq_sb.rearrange("p n s -> p (n s)"),
                                 lnq, AF.Exp, scale=-0.5)
            nc.scalar.activation(rss_sb.rearrange("p n s -> p (n s)"),
                                 lns, AF.Exp, scale=-0.5)

        # normalized tensors (rounded to f32r: they feed f32r matmuls)
        for k in range(KC):
            nc.gpsimd.tensor_tensor(snq_sb[:, k].bitcast(f32r), qv[:, k],
                                    rsq_sb, ALU.mult)
            nc.vector.tensor_tensor(sns_sb[:, k].bitcast(f32r), sv[:, k],
                                    rss_sb, ALU.mult)

        # transposes of x_se -> [spatial, C] layout for sp/qp matmuls
        with tc.tile_pool(name="ps_xt", bufs=4, space="PSUM") as PX:
            for t in range(NQL + WAY):
                for k in range(KC):
                    xt = PX.tile([HW, 128], f32, tag="xt")
                    if t < NQL:
                        src = qv[:, k, t, :]
                        dst = xTq_sb[:, t, 128 * k:128 * (k + 1)]
                    else:
                        src = sv[:, k, t - NQL, :]
                        dst = xTs_sb[:, t - NQL, 128 * k:128 * (k + 1)]
                    nc.tensor.transpose(xt, src, ident)
                    if t % 2 == 0:
                        nc.scalar.copy(dst, xt)
                    else:
                        nc.vector.tensor_copy(dst, xt)

        # ================= correlation + dual softmax =================
        snq = snq_sb
        sns = sns_sb
        with tc.tile_pool(name="ps_corr", bufs=4, space="PSUM") as PC, \
             tc.tile_pool(name="ps_attn", bufs=2, space="PSUM") as PA:

            qs_tiles = []
            for n in range(NQL):
                cq_ps = PC.tile([HW, WAY, HW], f32, tag="corr")   # 1 bank
                for k in range(KC):
                    nc.tensor.matmul(cq_ps.rearrange("p w s -> p (w s)"),
                                     snq[:, k, n, :].bitcast(f32r),
                                     sns[:, k].rearrange("p w s -> p (w s)").bitcast(f32r),
                                     start=(k == 0), stop=(k == KC - 1))
                qs_tiles.append(cq_ps)
                _softmax_tile(nc, S, f32, AF, ALU, cq_ps, WAY,
                              alphaq_sb[:, n, :], rq_sb[:, n, :],
                              expq_sb[:, n], k2c[:HW])

            sq_tiles = []
            for w in range(WAY):
                halves = []
                for h in range(2):
                    cs_ps = PC.tile([HW, 5, HW], f32, tag="corr")
                    for k in range(KC):
                        nc.tensor.matmul(
                            cs_ps.rearrange("p n s -> p (n s)"),
                            sns[:, k, w, :].bitcast(f32r),
                            snq[:, k, 5 * h:5 * h + 5, :].rearrange(
                                "p n s -> p (n s)").bitcast(f32r),
                            start=(k == 0), stop=(k == KC - 1))
                    halves.append(cs_ps)
                    _softmax_tile(nc, S, f32, AF, ALU, cs_ps, 5,
                                  alphas_sb[:, w, 5 * h:5 * h + 5],
                                  rs_sb[:, w, 5 * h:5 * h + 5],
                                  exps_sb[:, w, 5 * h:5 * h + 5],
                                  k2c[:HW])
                sq_tiles.append(halves)

            # attention sums via tiny matmuls:
            # attn_sT[s, (w,n)] = sum_l expq[n][l, w, s] * rq[n][l, w]
            for n in range(NQL):
                a_ps = PA.tile([HW, WAY], f32, tag="as")
                for w in range(WAY):
                    nc.tensor.matmul(a_ps[:, w:w + 1],
                                     expq_sb[:, n, w, :],
                                     rq_sb[:, n, w:w + 1],
                                     start=True, stop=True)
                nc.scalar.copy(attn_sT_sb[:, :, n], a_ps)
            # attn_qT[l, (n,w)] = sum_s exps[w][s, n, l] * rs[w][s, n]
            for w in range(WAY):
                a_ps = PA.tile([HW, NQL], f32, tag="aq")
                for n in range(NQL):
                    nc.tensor.matmul(a_ps[:, n:n + 1],
                                     exps_sb[:, w, n, :],
                                     rs_sb[:, w, n:n + 1],
                                     start=True, stop=True)
                nc.vector.tensor_copy(attn_qT_sb[:, :, w], a_ps)

        # ================= pooled prototypes + cosine =================
        # sp/qp with channels on partitions: out[c_chunk, q|w] per (w|n)
        with tc.tile_pool(name="ps_sp", bufs=2, space="PSUM") as PB:
            for w in range(WAY):
                b_ps = PB.tile([128, KC, NQL], f32, tag="sp")
                for j in range(KC):
                    nc.tensor.matmul(b_ps[:, j, :],
                                     xTs_sb[:, w, 128 * j:128 * (j + 1)],
                                     attn_sT_sb[:, w, :],
                                     start=True, stop=True)
                nc.scalar.copy(sp_all[:, :, :, w], b_ps)
            for n in range(NQL):
                c_ps = PB.tile([128, KC, WAY], f32, tag="qp")
                for j in range(KC):
                    nc.tensor.matmul(c_ps[:, j, :],
                                     xTq_sb[:, n, 128 * j:128 * (j + 1)],
                                     attn_qT_sb[:, n, :],
                                     start=True, stop=True)
                nc.vector.tensor_copy(qp_all[:, :, n, :], c_ps)

        # cosine: reduce over channels via ones-matmul, then over chunks on DVE
        with tc.tile_pool(name="ps_red", bufs=1, space="PSUM") as PR:
            red_ps = PR.tile([1, 3, 512], f32, tag="red")
            o1c = ones1[:, 0:1]
            spf = sp_all.rearrange("p k n w -> p (k n w)")
            qpf = qp_all.rearrange("p k n w -> p (k n w)")
            nc.vector.tensor_tensor(prod_sb, spf, qpf, ALU.mult)
            nc.tensor.matmul(red_ps[:, 0, 0:250], o1c,
                             prod_sb, start=True, stop=True)
            nc.vector.tensor_tensor(prod_sb, spf, spf, ALU.mult)
            nc.tensor.matmul(red_ps[:, 1, 0:250], o1c,
                             prod_sb, start=True, stop=True)
            nc.vector.tensor_tensor(prod_sb, qpf, qpf, ALU.mult)
            nc.tensor.matmul(red_ps[:, 2, 0:250], o1c,
                             prod_sb, start=True, stop=True)
            for i in range(3):
                nc.vector.tensor_reduce(
                    fin_sb[:, i, :],
                    red_ps[:, i, 0:250].rearrange("p (k nw) -> p nw k", k=KC),
                    AX.X, ALU.add)

        red = fin_sb
        nc.vector.tensor_tensor(red[:, 3, :], red[:, 1, :], red[:, 2, :],
                                ALU.mult)
        nc.scalar.activation(red[:, 4, :], red[:, 3, :], AF.Ln,
                             bias=epsc[:1], scale=1.0)
        nc.scalar.activation(red[:, 5, :], red[:, 4, :], AF.Exp, scale=-0.5)
        # sim = num * rden * (1/TEMP)
        nc.vector.scalar_tensor_tensor(red[:, 6, :], red[:, 0, :], 5.0,
                                       red[:, 5, :], ALU.mult, ALU.mult)
        nc.sync.dma_start(out_d.rearrange("a b -> b a"), red[:, 6, :])


def _softmax_tile(nc, S, f32, AF, ALU, corr_ps, nseg, alpha, r, exp_out,
                  k2_ap):
    """Row-softmax with gaussian normalization over the innermost axis.

    corr_ps: PSUM [100, nseg, 100]. For each (partition, segment) row:
      alpha = 1/(5*sqrt(var*100/99 + 1e-5)) = exp(-0.5*ln(var*K1 + K2))
      exp_out = exp(x * alpha)        (mean shift cancels in softmax)
      r = 1/sum(exp_out)
    """
    cp = corr_ps
    st6 = S.tile([HW, nseg, 6], f32, tag="st6")
    mv = S.tile([HW, nseg, 2], f32, tag="mv")
    for j in range(nseg):
        nc.vector.bn_stats(st6[:, j, :], cp[:, j, :])
        nc.vector.bn_aggr(mv[:, j, :], st6[:, j, :])
    lnv = S.tile([HW, nseg], f32, tag="lnv")
    nc.scalar.activation(lnv, mv[:, :, 1], AF.Ln,
                         bias=k2_ap, scale=K1)
    nc.scalar.activation(alpha, lnv, AF.Exp, scale=-0.5)
    rows = S.tile([HW, nseg], f32, tag="rows")
    for j in range(nseg):
        nc.scalar.activation(exp_out[:, j, :], cp[:, j, :], AF.Exp,
                             scale=alpha[:, j:j + 1],
                             accum_out=rows[:, j:j + 1])
    nc.vector.reciprocal(r, rows)


_NC_CACHE = {}


def _get_nc():
    if "nc" not in _NC_CACHE:
        _NC_CACHE["nc"] = _build()
    return _NC_CACHE["nc"]


def kernel(spt, qry, se_w1, se_b1, se_w2, se_b2):
    from concourse.bass_utils import run_bass_kernel_spmd

    spt = np.ascontiguousarray(
        np.asarray(spt, np.float32).reshape(WAY, C, HW))
    qry = np.asarray(qry, np.float32).reshape(NQ, C, HW)
    w1t = np.ascontiguousarray(np.asarray(se_w1, np.float32).T)   # [640, 40]
    w2t = np.ascontiguousarray(np.asarray(se_w2, np.float32).T)   # [40, 640]
    b1 = np.ascontiguousarray(np.asarray(se_b1, np.float32).reshape(40, 1))
    nb2 = np.ascontiguousarray(-np.asarray(se_b2, np.float32))

    shards = []
    for i in range(N_CORES - 1):
        shards.append(np.ascontiguousarray(qry[NQL * i: NQL * (i + 1)]))
    shards.append(np.ascontiguousarray(
        np.concatenate([qry[70:75], qry[70:75]], axis=0)))

    in_maps = [
        {"qry": shards[i], "spt": spt, "w1t": w1t, "w2t": w2t,
         "b1": b1, "nb2": nb2}
        for i in range(N_CORES)
    ]
    nc = _get_nc()
    res = run_bass_kernel_spmd(nc, in_maps, core_ids=list(range(N_CORES)))
    outs = []
    for i in range(N_CORES):
        o = np.asarray(res.results[i]["out"]).reshape(NQL, WAY)  # [10, 5]
        outs.append(o[:5] if i == N_CORES - 1 else o)
    return np.concatenate(outs, axis=0).astype(np.float32)


if __name__ == "__main__":
    pass


# revision 16
# speedup vs baseline: 1.4233x; 1.4233x over previous
"""Trainium2 Bass kernel for DCANet head (nn_DCANet_1795296329879).

Data-parallel over num_qry: 75 queries -> 8 cores x 10 (core 7 padded with
duplicates). Support features + SE weights replicated. No collectives.

Per core (NQL=10 local queries, WAY=5, C=640, HW=100):
  x = x - mean_c(x);  x *= SE_gate(x);  n = x/||x||_c
  corr[q,w,s,l] = <n_spt[w,:,s], n_qry[q,:,l]>        (bf16 ops, f32 acc)
  cs = softmax_s(gauss_norm_s(corr)/5); attn_s = sum_l cs
  cq = softmax_l(gauss_norm_l(corr)/5); attn_q = sum_s cq
  sp = spt @ attn_s; qp = qry @ attn_q; out = cos(sp,qp)/0.2

Implementation notes:
 - Heavy tensors in bf16 (measured end-to-end rel err ~5e-3 vs 2e-2 gate).
 - Softmax alpha (inv std) computed in a single batched Ln/Exp pass to
   avoid ACT table-set thrash (Ln and Exp live in different default sets).
 - Correlation PSUM tiles are copied to SBUF bf16 immediately so the PE
   can stream all 100 corr matmuls back to back.
 - x_se stored spatially padded to 128 so the xbar DMA-transpose
   (16-bit, free%128==0) can produce the [spatial, C] layouts.
"""

import numpy as np

NQL = 10          # queries per core
WAY = 5
C = 640
KC = 5            # C = KC * 128 chunks
HW = 100
HWP = 128         # padded spatial for DMA transpose
NQ = 75
N_CORES = 8

K1 = 2500.0 / 99.0    # 25 * (100/99): unbiased var + /TEMP_ATTN^2
K2 = 2.5e-4           # 25 * eps(1e-5)


def _build():
    import concourse.bass as bass
    import concourse.tile as tile
    from concourse import bacc, mybir

    f32 = mybir.dt.float32
    bf16 = mybir.dt.bfloat16
    AF = mybir.ActivationFunctionType
    ALU = mybir.AluOpType
    AX = mybir.AxisListType

    nc = bacc.Bacc("TRN2", target_bir_lowering=False, debug=False,
                   num_devices=N_CORES)

    qry_d = nc.dram_tensor("qry", [NQL, C, HW], f32, kind="ExternalInput").ap()
    spt_d = nc.dram_tensor("spt", [WAY, C, HW], f32, kind="ExternalInput").ap()
    w1t_d = nc.dram_tensor("w1t", [C, 40], f32, kind="ExternalInput").ap()
    w2t_d = nc.dram_tensor("w2t", [40, C], f32, kind="ExternalInput").ap()
    b1_d = nc.dram_tensor("b1", [40, 1], f32, kind="ExternalInput").ap()
    nb2_d = nc.dram_tensor("nb2", [C], f32, kind="ExternalInput").ap()
    out_d = nc.dram_tensor("out", [NQL * WAY, 1], f32, kind="ExternalOutput").ap()

    with tile.TileContext(nc) as tc:
        _body(nc, tc, mybir, f32, bf16, AF, ALU, AX,
              qry_d, spt_d, w1t_d, w2t_d, b1_d, nb2_d, out_d)
    nc.compile()
    return nc


def _body(nc, tc, mybir, f32, bf16, AF, ALU, AX,
          qry_d, spt_d, w1t_d, w2t_d, b1_d, nb2_d, out_d):
    from contextlib import ExitStack

    ctx = ExitStack()
    with ctx:
        P = ctx.enter_context(tc.tile_pool(name="persist", bufs=1))
        S = ctx.enter_context(tc.tile_pool(name="scratch", bufs=3))

        # ---- persistent SBUF tiles ----
        qv = P.tile([128, KC, NQL, HW], f32, name="qv")
        sv = P.tile([128, KC, WAY, HW], f32, name="sv")
        qbf = P.tile([128, KC, NQL, HW], bf16, name="qbf")   # raw bf16
        sbf = P.tile([128, KC, WAY, HW], bf16, name="sbf")
        xq = P.tile([128, KC, NQL, HWP], bf16, name="xq")    # centered->x_se
        xs = P.tile([128, KC, WAY, HWP], bf16, name="xs")
        snq = P.tile([128, KC, NQL, HW], bf16, name="snq")   # normalized
        sns = P.tile([128, KC, WAY, HW], bf16, name="sns")
        w1t_sb = P.tile([128, KC, 40], f32, name="w1t_sb")
        w2t_sb = P.tile([40, C], f32, name="w2t_sb")
        b1_sb = P.tile([40, 1], f32, name="b1_sb")
        nb2_sb = P.tile([128, KC], f32, name="nb2_sb")
        mbq = P.tile([128, NQL, HW], f32, name="mbq")        # chan means
        mbs = P.tile([128, WAY, HW], f32, name="mbs")
        rsq = P.tile([128, NQL, HW], f32, name="rsq")        # 1/||x||
        rss = P.tile([128, WAY, HW], f32, name="rss")
        sall = P.tile([128, KC, NQL + WAY], f32, name="sall")
        g_sb = P.tile([128, KC, NQL + WAY], f32, name="g_sb")
        h_sb = P.tile([40, NQL + WAY], f32, name="h_sb")
        xTq = P.tile([128, NQL, C], bf16, name="xTq")        # [l, n, c]
        xTs = P.tile([128, WAY, C], bf16, name="xTs")        # [s, w, c]
        # corr SBUF copies (bf16)
        cq_all = [P.tile([HW, NQL, HW], bf16, name=f"cqt{t}") for t in range(5)]
        cs_all = [P.tile([HW, NQL, HW], bf16, name=f"cst{w}") for w in range(WAY)]
        varq = P.tile([HW, NQL, WAY], f32, name="varq")      # 100*var_b
        vars_ = P.tile([HW, WAY, NQL], f32, name="vars_")
        alq = P.tile([HW, NQL, WAY], bf16, name="alq")       # alpha (bf16)
        als = P.tile([HW, WAY, NQL], bf16, name="als")
        attn_sT = P.tile([HW, WAY, NQL], bf16, name="attn_sT")
        attn_qT = P.tile([HW, NQL, WAY], bf16, name="attn_qT")
        sp_all = P.tile([128, KC, NQL, WAY], f32, name="sp_all")
        qp_all = P.tile([128, KC, NQL, WAY], f32, name="qp_all")
        prod = P.tile([128, KC * NQL * WAY], f32, name="prod")
        fin = P.tile([1, 8, NQL * WAY], f32, name="fin")
        onesbf = P.tile([128, 128], bf16, name="onesbf")
        ones1 = P.tile([128, 128], f32, name="ones1")
        epsc = P.tile([128, 1], f32, name="epsc")
        k2t = P.tile([128, 1], f32, name="k2t")

        nc.vector.memset(onesbf, 1.0)
        nc.vector.memset(ones1, 1.0)
        nc.vector.memset(epsc, 1e-30)
        nc.vector.memset(k2t, K2)
        # zero the spatial pad of x_se (read by the DMA transposes)
        nc.gpsimd.memset(xq[:, :, :, HW:], 0.0)
        nc.gpsimd.memset(xs[:, :, :, HW:], 0.0)

        # ---- input DMAs ----
        qry_dv = qry_d.rearrange("n (k p) s -> p k n s", p=128)
        spt_dv = spt_d.rearrange("n (k p) s -> p k n s", p=128)
        for k in range(KC):
            nc.sync.dma_start(qv[:, k], qry_dv[:, k])
            nc.sync.dma_start(sv[:, k], spt_dv[:, k])
        nc.sync.dma_start(w1t_sb, w1t_d.rearrange("(k p) j -> p k j", p=128))
        nc.sync.dma_start(w2t_sb, w2t_d)
        nc.sync.dma_start(b1_sb, b1_d)
        nc.sync.dma_start(nb2_sb, nb2_d.rearrange("(k p) -> p k", p=128))

        # ================= preprocessing =================
        # bf16 shadows of raw x (ACT for qry, DVE for spt)
        for k in range(KC):
            nc.scalar.copy(qbf[:, k], qv[:, k])
            nc.vector.tensor_copy(sbf[:, k], sv[:, k])

        # channel-sum via ones matmul (bf16), scaled 1/C on PSUM->SBUF copy
        with tc.tile_pool(name="ps_pre", bufs=1, space="PSUM") as PP:
            mbq_ps = [PP.tile([128, 500], f32, tag=f"mbq{h}", name=f"mbq_ps{h}")
                      for h in range(2)]
            mbs_ps = PP.tile([128, WAY * HW], f32, tag="mbs", name="mbs_ps")
            for h in range(2):
                for k in range(KC):
                    nc.tensor.matmul(mbq_ps[h], onesbf,
                                     qbf[:, k, 5 * h:5 * h + 5, :],
                                     start=(k == 0), stop=(k == KC - 1))
            for k in range(KC):
                nc.tensor.matmul(mbs_ps, onesbf, sbf[:, k, :, :],
                                 start=(k == 0), stop=(k == KC - 1))
            mbq_f = mbq.rearrange("p n s -> p (n s)")
            nc.scalar.mul(mbq_f[:, 0:500], mbq_ps[0], 1.0 / C)
            nc.scalar.mul(mbq_f[:, 500:1000], mbq_ps[1], 1.0 / C)
            nc.vector.tensor_scalar_mul(
                mbs.rearrange("p n s -> p (n s)"), mbs_ps, 1.0 / C)

            # centered x in bf16 (into padded tiles)
            for k in range(KC):
                nc.gpsimd.tensor_tensor(xq[:, k, :, :HW], qv[:, k], mbq,
                                        ALU.subtract)
                nc.vector.tensor_tensor(xs[:, k, :, :HW], sv[:, k], mbs,
                                        ALU.subtract)

        # SE: spatial sum -> fc1 -> relu -> fc2 -> sigmoid
        nc.vector.tensor_reduce(sall[:, :, :NQL], xq[:, :, :, :HW], AX.X,
                                ALU.add)
        nc.vector.tensor_reduce(sall[:, :, NQL:], xs[:, :, :, :HW], AX.X,
                                ALU.add)
        with tc.tile_pool(name="ps_se", bufs=1, space="PSUM") as PS:
            h_ps = PS.tile([40, NQL + WAY], f32, tag="h", name="h_ps")
            g_ps = PS.tile([128, KC, NQL + WAY], f32, tag="g", name="g_ps")
            for k in range(KC):
                nc.tensor.matmul(h_ps, w1t_sb[:, k, :], sall[:, k, :],
                                 start=(k == 0), stop=(k == KC - 1))
            nc.scalar.activation(h_sb, h_ps, AF.Relu,
                                 bias=b1_sb[:, 0:1], scale=1.0 / HW)
            for k in range(KC):
                nc.tensor.matmul(g_ps[:, k, :],
                                 w2t_sb[:, 128 * k:128 * (k + 1)],
                                 h_sb, start=True, stop=True)
                # sigmoid(x) = 1/(1+exp(-x)); bias holds -b2
                nc.scalar.activation(g_sb[:, k, :], g_ps[:, k, :],
                                     AF.Exp, bias=nb2_sb[:, k:k + 1],
                                     scale=-1.0)
        nc.vector.tensor_scalar_add(g_sb, g_sb, 1.0)
        nc.vector.reciprocal(g_sb, g_sb)

        # apply SE gates in place on the padded bf16 tiles
        for k in range(KC):
            nc.gpsimd.tensor_tensor(
                xq[:, k, :, :HW], xq[:, k, :, :HW],
                g_sb[:, k, :NQL].to_broadcast((128, NQL, HW)), ALU.mult)
            nc.vector.tensor_tensor(
                xs[:, k, :, :HW], xs[:, k, :, :HW],
                g_sb[:, k, NQL:].to_broadcast((128, WAY, HW)), ALU.mult)

        # L2 norms over channels; rs = exp(-0.5 ln ss)
        with tc.tile_pool(name="ps_ss", bufs=1, space="PSUM") as PN:
            ssq_ps = [PN.tile([128, 500], f32, tag=f"ssq{h}", name=f"ssq_ps{h}")
                      for h in range(2)]
            sss_ps = PN.tile([128, WAY * HW], f32, tag="sss", name="sss_ps")
            for k in range(KC):
                x2q = S.tile([128, NQL, HW], bf16, tag="x2q", name="x2q")
                nc.vector.tensor_tensor(x2q, xq[:, k, :, :HW],
                                        xq[:, k, :, :HW], ALU.mult)
                for h in range(2):
                    nc.tensor.matmul(
                        ssq_ps[h], onesbf,
                        x2q.rearrange("p n s -> p (n s)")[:, h * 500:(h + 1) * 500],
                        start=(k == 0), stop=(k == KC - 1))
                x2s = S.tile([128, WAY, HW], bf16, tag="x2s", name="x2s")
                nc.gpsimd.tensor_tensor(x2s, xs[:, k, :, :HW],
                                        xs[:, k, :, :HW], ALU.mult)
                nc.tensor.matmul(sss_ps, onesbf,
                                 x2s.rearrange("p n s -> p (n s)"),
                                 start=(k == 0), stop=(k == KC - 1))
            lnq = S.tile([128, NQL * HW], f32, tag="lnq", name="lnq")
            lns = S.tile([128, WAY * HW], f32, tag="lns", name="lns")
            for h in range(2):
                nc.scalar.activation(lnq[:, 500 * h:500 * (h + 1)], ssq_ps[h],
                                     AF.Ln, bias=epsc, scale=1.0)
            nc.scalar.activation(lns, sss_ps, AF.Ln, bias=epsc, scale=1.0)
            nc.scalar.activation(rsq.rearrange("p n s -> p (n s)"), lnq,
                                 AF.Exp, scale=-0.5)
            nc.scalar.activation(rss.rearrange("p n s -> p (n s)"), lns,
                                 AF.Exp, scale=-0.5)

        # normalized tensors (bf16)
        for k in range(KC):
            nc.gpsimd.tensor_tensor(snq[:, k], xq[:, k, :, :HW], rsq, ALU.mult)
            nc.vector.tensor_tensor(sns[:, k], xs[:, k, :, :HW], rss, ALU.mult)

        # x_se -> [spatial, C] layouts via xbar DMA transpose (bf16)
        for t in range(NQL + WAY):
            for k in range(KC):
                if t < NQL:
                    src = xq[:, k, t, :]
                    dst = xTq[:, t, 128 * k:128 * (k + 1)]
                else:
                    src = xs[:, k, t - NQL, :]
                    dst = xTs[:, t - NQL, 128 * k:128 * (k + 1)]
                eng = nc.sync if (t + k) % 2 == 0 else nc.scalar
                eng.dma_start_transpose(dst, src)

        # ================= correlation (dense PE phase) =================
        # 10 two-bank PSUM tiles; each copied to SBUF bf16 right away.
        with tc.tile_pool(name="ps_corr", bufs=3, space="PSUM") as PC, \
             tc.tile_pool(name="ps_attn", bufs=2, space="PSUM") as PA:
            for t in range(5):          # qs tiles: queries (2t, 2t+1)
                c_ps = PC.tile([HW, 2, 512], f32, tag="corr", name="c_ps")
                for j in range(2):
                    n = 2 * t + j
                    for k in range(KC):
                        nc.tensor.matmul(
                            c_ps[:, j, 0:500],
                            snq[:, k, n, :],
                            sns[:, k].rearrange("p w s -> p (w s)"),
                            start=(k == 0), stop=(k == KC - 1))
                nc.scalar.copy(
                    cq_all[t].rearrange("p (a w) s -> p a w s", a=2),
                    c_ps[:, :, 0:500].rearrange("p a (w s) -> p a w s", s=HW))
            for w in range(WAY):        # sq tiles
                c_ps = PC.tile([HW, 2, 512], f32, tag="corr", name="c_ps")
                for h in range(2):
                    for k in range(KC):
                        nc.tensor.matmul(
                            c_ps[:, h, 0:500],
                            sns[:, k, w, :],
                            snq[:, k, 5 * h:5 * h + 5, :].rearrange(
                                "p n s -> p (n s)"),
                            start=(k == 0), stop=(k == KC - 1))
                nc.vector.tensor_copy(
                    cs_all[w].rearrange("p (a n) s -> p a n s", a=2),
                    c_ps[:, :, 0:500].rearrange("p a (n s) -> p a n s", s=HW))

            # --- stats: sum / sum-of-squares reduces ---
            # 100*var_b = sum(x^2) - sum(x)^2/100
            jobs = [(cq_all[t],
                     varq.rearrange("p n w -> p (n w)")[:, 10 * t:10 * (t + 1)],
                     True) for t in range(5)]
            jobs += [(cs_all[w], vars_[:, w, :], False) for w in range(WAY)]
            for i, (cb, var, on_dve) in enumerate(jobs):
                sq = S.tile([HW, NQL, HW], bf16, tag="sq", name="sq")
                if on_dve:
                    nc.vector.tensor_tensor(sq, cb, cb, ALU.mult)
                else:
                    nc.gpsimd.tensor_tensor(sq, cb, cb, ALU.mult)
                s1 = S.tile([HW, NQL], f32, tag="s1", name="s1")
                s2 = S.tile([HW, NQL], f32, tag="s2", name="s2")
                nc.vector.tensor_reduce(s1, cb, AX.X, ALU.add)
                nc.vector.tensor_reduce(s2, sq, AX.X, ALU.add)
                nc.vector.tensor_tensor(s1, s1, s1, ALU.mult)
                nc.vector.scalar_tensor_tensor(var, s1, -0.01, s2,
                                               ALU.mult, ALU.add)

            # --- batched alpha = exp(-0.5 ln(var*K1/100 + K2)) ---
            lnv = S.tile([HW, NQL * WAY], f32, tag="lnv", name="lnv")
            lnv2 = S.tile([HW, NQL * WAY], f32, tag="lnv2", name="lnv2")
            nc.scalar.activation(lnv, varq.rearrange("p n w -> p (n w)"),
                                 AF.Ln, bias=k2t[:HW], scale=K1 / 100.0)
            nc.scalar.activation(lnv2, vars_.rearrange("p w n -> p (w n)"),
                                 AF.Ln, bias=k2t[:HW], scale=K1 / 100.0)
            nc.scalar.activation(alq.rearrange("p n w -> p (n w)"), lnv,
                                 AF.Exp, scale=-0.5)
            nc.scalar.activation(als.rearrange("p w n -> p (w n)"), lnv2,
                                 AF.Exp, scale=-0.5)

            # --- exp, rowsums, attn matmuls ---
            for t in range(5):          # qs tiles -> attn_sT columns
                zb = S.tile([HW, NQL, HW], bf16, tag="zb", name="zb")
                nc.vector.tensor_tensor(
                    zb, cq_all[t][:, :, 0:HW],
                    alq.rearrange("p n w -> p (n w)")[
                        :, 10 * t:10 * (t + 1)].to_broadcast((HW, NQL, HW)),
                    ALU.mult)
                eb = S.tile([HW, NQL, HW], bf16, tag="eb", name="eb")
                nc.scalar.activation(eb, zb, AF.Exp)
                rows = S.tile([HW, NQL], f32, tag="rows", name="rows")
                nc.vector.tensor_reduce(rows, eb, AX.X, ALU.add)
                nc.vector.reciprocal(rows, rows)
                rb = S.tile([HW, NQL], bf16, tag="rb", name="rb")
                nc.vector.tensor_copy(rb, rows)
                for j in range(2):
                    n = 2 * t + j
                    a_ps = PA.tile([HW, WAY], f32, tag="attn", name="a_ps")
                    for w in range(WAY):
                        nc.tensor.matmul(a_ps[:, w:w + 1],
                                         eb[:, 5 * j + w, :],
                                         rb[:, 5 * j + w:5 * j + w + 1],
                                         start=True, stop=True)
                    nc.scalar.copy(attn_sT[:, :, n], a_ps)
            for w in range(WAY):        # sq tiles -> attn_qT columns
                zb = S.tile([HW, NQL, HW], bf16, tag="zb", name="zb")
                nc.gpsimd.tensor_tensor(
                    zb, cs_all[w][:, :, 0:HW],
                    als[:, w, :].to_broadcast((HW, NQL, HW)), ALU.mult)
                eb = S.tile([HW, NQL, HW], bf16, tag="eb", name="eb")
                nc.scalar.activation(eb, zb, AF.Exp)
                rows = S.tile([HW, NQL], f32, tag="rows", name="rows")
                nc.vector.tensor_reduce(rows, eb, AX.X, ALU.add)
                nc.vector.reciprocal(rows, rows)
                rb = S.tile([HW, NQL], bf16, tag="rb", name="rb")
                nc.vector.tensor_copy(rb, rows)
                aq_ps = PA.tile([HW, NQL], f32, tag="attn", name="aq_ps")
                for n in range(NQL):
                    nc.tensor.matmul(aq_ps[:, n:n + 1], eb[:, n, :],
                                     rb[:, n:n + 1], start=True, stop=True)
                nc.vector.tensor_copy(attn_qT[:, :, w], aq_ps)

        # ================= pooled prototypes + cosine =================
        with tc.tile_pool(name="ps_sp", bufs=2, space="PSUM") as PB:
            for w in range(WAY):
                b_ps = PB.tile([128, KC, NQL], f32, tag="sp", name="b_ps")
                for j in range(KC):
                    nc.tensor.matmul(b_ps[:, j, :],
                                     xTs[:HW, w, 128 * j:128 * (j + 1)],
                                     attn_sT[:, w, :], start=True, stop=True)
                nc.scalar.copy(sp_all[:, :, :, w], b_ps)
            for n in range(NQL):
                c_ps2 = PB.tile([128, KC, WAY], f32, tag="qp", name="c_ps2")
                for j in range(KC):
                    nc.tensor.matmul(c_ps2[:, j, :],
                                     xTq[:HW, n, 128 * j:128 * (j + 1)],
                                     attn_qT[:, n, :], start=True, stop=True)
                nc.vector.tensor_copy(qp_all[:, :, n, :], c_ps2)

        with tc.tile_pool(name="ps_red", bufs=1, space="PSUM") as PR:
            red_ps = PR.tile([1, 3, 512], f32, tag="red", name="red_ps")
            o1c = ones1[:, 0:1]
            spf = sp_all.rearrange("p k n w -> p (k n w)")
            qpf = qp_all.rearrange("p k n w -> p (k n w)")
            nc.vector.tensor_tensor(prod, spf, qpf, ALU.mult)
            nc.tensor.matmul(red_ps[:, 0, 0:250], o1c, prod,
                             start=True, stop=True)
            nc.vector.tensor_tensor(prod, spf, spf, ALU.mult)
            nc.tensor.matmul(red_ps[:, 1, 0:250], o1c, prod,
                             start=True, stop=True)
            nc.vector.tensor_tensor(prod, qpf, qpf, ALU.mult)
            nc.tensor.matmul(red_ps[:, 2, 0:250], o1c, prod,
                             start=True, stop=True)
            for i in range(3):
                nc.vector.tensor_reduce(
                    fin[:, i, :],
                    red_ps[:, i, 0:250].rearrange("p (k nw) -> p nw k", k=KC),
                    AX.X, ALU.add)

        nc.vector.tensor_tensor(fin[:, 3, :], fin[:, 1, :], fin[:, 2, :],
                                ALU.mult)
        nc.scalar.activation(fin[:, 4, :], fin[:, 3, :], AF.Ln,
                             bias=epsc[:1], scale=1.0)
        nc.scalar.activation(fin[:, 5, :], fin[:, 4, :], AF.Exp, scale=-0.5)
        nc.vector.scalar_tensor_tensor(fin[:, 6, :], fin[:, 0, :], 5.0,
                                       fin[:, 5, :], ALU.mult, ALU.mult)
        nc.sync.dma_start(out_d.rearrange("a b -> b a"), fin[:, 6, :])


_NC_CACHE = {}


def _get_nc():
    if "nc" not in _NC_CACHE:
        _NC_CACHE["nc"] = _build()
    return _NC_CACHE["nc"]


def kernel(spt, qry, se_w1, se_b1, se_w2, se_b2):
    from concourse.bass_utils import run_bass_kernel_spmd

    spt = np.ascontiguousarray(
        np.asarray(spt, np.float32).reshape(WAY, C, HW))
    qry = np.asarray(qry, np.float32).reshape(NQ, C, HW)
    w1t = np.ascontiguousarray(np.asarray(se_w1, np.float32).T)   # [640, 40]
    w2t = np.ascontiguousarray(np.asarray(se_w2, np.float32).T)   # [40, 640]
    b1 = np.ascontiguousarray(np.asarray(se_b1, np.float32).reshape(40, 1))
    nb2 = np.ascontiguousarray(-np.asarray(se_b2, np.float32))

    shards = []
    for i in range(N_CORES - 1):
        shards.append(np.ascontiguousarray(qry[NQL * i: NQL * (i + 1)]))
    shards.append(np.ascontiguousarray(
        np.concatenate([qry[70:75], qry[70:75]], axis=0)))

    in_maps = [
        {"qry": shards[i], "spt": spt, "w1t": w1t, "w2t": w2t,
         "b1": b1, "nb2": nb2}
        for i in range(N_CORES)
    ]
    nc = _get_nc()
    res = run_bass_kernel_spmd(nc, in_maps, core_ids=list(range(N_CORES)))
    outs = []
    for i in range(N_CORES):
        o = np.asarray(res.results[i]["out"]).reshape(NQL, WAY)
        outs.append(o[:5] if i == N_CORES - 1 else o)
    return np.concatenate(outs, axis=0).astype(np.float32)


if __name__ == "__main__":
    pass
